# revision 15
# baseline (speedup 1.0000x reference)
"""AnomalyAttention distributed Bass kernel for 8 TRN2 NeuronCores.

Shards the 32 (batch, head) pairs across 8 cores (4 pairs/core), no
collectives. Per pair it computes causal softmax attention (series, V),
a per-row Gaussian prior, and the broadcast sigma tensor.

Layout strategy (all host-side prep is pure numpy reshuffling):
  - Q, K ship as bf16 pre-transposed to [pair, E, L] so the TensorE
    contraction dim (E=64) lands on SBUF partitions with contiguous
    DMAs; scores accumulate in f32 PSUM (fp32 matmul on TRN2 runs as
    LOW/HIGH double passes with no fast-weight-load -- 8x slower).
  - V ships as bf16 chunk-major [pair, 128, 8*64] so each 128-row
    s-chunk is a [128, 64] SBUF slice.
  - sigma is pre-transformed on host into per-row (sg, -1/(2 sg^2),
    ln(1/(sqrt(2pi) sg))) so prior is a single ScalarE
    Exp(d^2 * scale + bias) pass over a shifted window of a static
    distance^2 table.
  - series is only written on the causal half; the runtime pre-zeros
    output buffers, so the strict upper triangle stays 0.
  - The P^T needed by the P@V matmul comes from PE transposes of a
    bf16 copy of the exp'd tile (cast on the otherwise idle GpSimd),
    batched 4 chunks per PSUM bank with one VectorE copy per group.
"""

import sys

if "/opt/trn_rl_repo" not in sys.path:
    sys.path.insert(0, "/opt/trn_rl_repo")

import ml_dtypes
import numpy as np

_B, _L, _H, _E, _D = 4, 1024, 8, 64, 64
_NCORES = 8
_PAIRS = (_B * _H) // _NCORES  # 4 (b,h) pairs per core
_P = 128
_NT = _L // _P  # 8 row tiles of 128

_BF16 = ml_dtypes.bfloat16

LAST_EXEC_NS = None
LAST_RESULTS = None

_state = {"nc": None}


def _build_bass():
    import concourse.bacc as bacc
    import concourse.mybir as mybir
    from concourse.tile import TileContext

    f32 = mybir.dt.float32
    bf16 = mybir.dt.bfloat16
    AF = mybir.ActivationFunctionType
    ALU = mybir.AluOpType

    nc = bacc.Bacc()
    qT_ext = nc.declare_dram_parameter("qT", [_PAIRS, _E, _L], bf16, isOutput=False)
    kT_ext = nc.declare_dram_parameter("kT", [_PAIRS, _E, _L], bf16, isOutput=False)
    v_ext = nc.declare_dram_parameter(
        "v", [_PAIRS, _P, _NT * _D], bf16, isOutput=False
    )
    vec_ext = nc.declare_dram_parameter(
        "vecs", [_P, _PAIRS, _NT, 3], f32, isOutput=False
    )
    g_ext = nc.declare_dram_parameter("gsq", [_P, 2 * _L], f32, isOutput=False)
    mA_ext = nc.declare_dram_parameter("maskA", [_P, _P], f32, isOutput=False)
    mT_ext = nc.declare_dram_parameter("maskT", [_P, _P], f32, isOutput=False)

    ser_ext = nc.declare_dram_parameter("ser", [_PAIRS, _L, _L], bf16, isOutput=True)
    pri_ext = nc.declare_dram_parameter("pri", [_PAIRS, _L, _L], bf16, isOutput=True)
    sig_ext = nc.declare_dram_parameter("sig", [_PAIRS, _L, _L], bf16, isOutput=True)
    vout_ext = nc.declare_dram_parameter("vout", [_PAIRS, _L, _D], f32, isOutput=True)

    with TileContext(nc) as tc:
        with (
            tc.tile_pool(name="consts", bufs=1) as consts,
            tc.tile_pool(name="pairin", bufs=2) as pairin,
            tc.tile_pool(name="work", bufs=3) as work,
            tc.tile_pool(name="etp", bufs=2) as etp,
            tc.tile_pool(name="small", bufs=4) as small,
            tc.tile_pool(name="psA", bufs=2, space="PSUM") as psA_pool,
            tc.tile_pool(name="psB", bufs=2, space="PSUM") as psB_pool,
            tc.tile_pool(name="psVa", bufs=1, space="PSUM") as psVa_pool,
            tc.tile_pool(name="psVb", bufs=1, space="PSUM") as psVb_pool,
        ):
            g_t = consts.tile([_P, 2 * _L], f32)
            nc.sync.dma_start(g_t[:], g_ext[:])
            mA_t = consts.tile([_P, _P], f32)
            nc.sync.dma_start(mA_t[:], mA_ext[:])
            mT_t = consts.tile([_P, _P], f32)
            nc.sync.dma_start(mT_t[:], mT_ext[:])
            vec_t = consts.tile([_P, _PAIRS, _NT, 3], f32)
            nc.sync.dma_start(vec_t[:], vec_ext[:])
            zb_t = consts.tile([_P, _L], bf16)
            nc.gpsimd.memset(zb_t[:], 0.0)

            for p in range(_PAIRS):
                q_t = pairin.tile([_E, _L], bf16, tag="q")
                nc.sync.dma_start(q_t[:], qT_ext[p])
                k_t = pairin.tile([_E, _L], bf16, tag="k")
                nc.sync.dma_start(k_t[:], kT_ext[p])
                v_t = pairin.tile([_P, _NT * _D], bf16, tag="v")
                nc.sync.dma_start(v_t[:], v_ext[p])

                for kk in range(_NT // 2):
                    t0, t1 = 2 * kk, 2 * kk + 1
                    m0g = t0 * _P  # row base of the 2-tile group
                    w0, w1 = (t0 + 1) * _P, (t1 + 1) * _P
                    rows0 = slice(t0 * _P, (t0 + 1) * _P)
                    rows1 = slice(t1 * _P, (t1 + 1) * _P)

                    # ---- PE block: row scores for both tiles ----
                    psA0 = psA_pool.tile([_P, _L], f32, tag="psA")
                    for n0 in range(0, w0, 512):
                        n1 = min(w0, n0 + 512)
                        nc.tensor.matmul(
                            psA0[:, n0:n1], q_t[:, rows0], k_t[:, n0:n1],
                            start=True, stop=True,
                        )
                    psA1 = psA_pool.tile([_P, _L], f32, tag="psA")
                    for n0 in range(0, w1, 512):
                        n1 = min(w1, n0 + 512)
                        nc.tensor.matmul(
                            psA1[:, n0:n1], q_t[:, rows1], k_t[:, n0:n1],
                            start=True, stop=True,
                        )

                    # ---- PE block: scores^T chunks, N=256 over both tiles
                    # (last chunk t1 is diag-only, N=128), 2 chunks per bank
                    et_t = etp.tile([_P, _NT, 2 * _P], bf16, tag="et")
                    psBs = []
                    for u0 in range(0, t1 + 1, 2):
                        psB = psB_pool.tile([_P, 4 * _P], f32, tag="psB")
                        nc.tensor.matmul(
                            psB[:, 0 : 2 * _P],
                            k_t[:, u0 * _P : (u0 + 1) * _P],
                            q_t[:, m0g : m0g + 2 * _P],
                            start=True, stop=True,
                        )
                        if u0 + 1 == t1:
                            nc.tensor.matmul(
                                psB[:, 3 * _P : 4 * _P],
                                k_t[:, t1 * _P : (t1 + 1) * _P],
                                q_t[:, rows1],
                                start=True, stop=True,
                            )
                        else:
                            nc.tensor.matmul(
                                psB[:, 2 * _P : 4 * _P],
                                k_t[:, (u0 + 1) * _P : (u0 + 2) * _P],
                                q_t[:, m0g : m0g + 2 * _P],
                                start=True, stop=True,
                            )
                        psBs.append(psB)

                    # diagonal masks: chunk t0 at slot cols 0:128 of the last
                    # bank's first slot; chunk t1 at slot cols 128:256 of its
                    # second slot
                    psB_last = psBs[-1]
                    nc.vector.tensor_add(
                        psB_last[:, 0:_P], psB_last[:, 0:_P], mT_t[:]
                    )
                    nc.vector.tensor_add(
                        psB_last[:, 3 * _P : 4 * _P],
                        psB_last[:, 3 * _P : 4 * _P],
                        mT_t[:],
                    )
                    for i, psB in enumerate(psBs):
                        nc.scalar.activation(
                            et_t[:, 2 * i : 2 * i + 2, :],
                            psB[:, : 4 * _P],
                            AF.Exp,
                            scale=0.125,
                        )

                    # ---- per-tile row path + V ----
                    sig_t = work.tile([_P, 2, _L], bf16, tag="sig")
                    vo_t = small.tile([_P, 2, _D], f32, tag="vo")
                    for ti, (t, psA, w, rows) in enumerate(
                        ((t0, psA0, w0, rows0), (t1, psA1, w1, rows1))
                    ):
                        # causal mask on the diagonal 128x128 block
                        nc.vector.tensor_add(
                            psA[:, t * _P : w], psA[:, t * _P : w], mA_t[:]
                        )
                        # exp(scale * scores) -> bf16 + per-row sums
                        eb_t = work.tile([_P, _L], bf16, tag="eb")
                        rsum = small.tile([_P, 1], f32, tag="rsum")
                        nc.scalar.activation(
                            eb_t[:, :w], psA[:, :w], AF.Exp,
                            scale=0.125, accum_out=rsum[:],
                        )
                        recip = small.tile([_P, 1], f32, tag="recip")
                        nc.vector.reciprocal(recip[:], rsum[:])
                        s_t = work.tile([_P, _L], bf16, tag="s")
                        nc.vector.tensor_scalar_mul(
                            s_t[:, :w], eb_t[:, :w], recip[:]
                        )
                        nc.sync.dma_start(ser_ext[p, rows, 0:w], s_t[:, :w])

                        # prior: sg <= 3^1-1 ~= 2.0 always, so the f32 result
                        # underflows to exactly 0 for |i-j| >= 29 (in the
                        # reference too) -- only a band around the diagonal
                        # needs computing/writing (padded to 256 cols for
                        # >=512B DMA rows); pre-zeroed output covers the rest
                        m0 = t * _P
                        lo = max(0, m0 - 64)
                        hi = min(_L, lo + 2 * _P)
                        bw = hi - lo
                        pri_t = work.tile([_P, 2 * _P], bf16, tag="pri")
                        gview = g_t[:, lo - m0 + _L : hi - m0 + _L]
                        nc.scalar.activation(
                            pri_t[:, :bw], gview, AF.Exp,
                            scale=vec_t[:, p, t, 1:2],
                            bias=vec_t[:, p, t, 2:3],
                        )
                        nc.sync.dma_start(
                            pri_ext[p, rows, lo:hi], pri_t[:, :bw]
                        )

                        # sigma_out = sg broadcast along the free dim
                        nc.vector.tensor_scalar(
                            sig_t[:, ti, :], zb_t[:], 0.0,
                            vec_t[:, p, t, 0:1], ALU.mult, ALU.add,
                        )

                        # V: accumulate P^T-chunk matmuls, alternating two
                        # PSUM banks so consecutive matmuls pipeline
                        mcol = ti * _P
                        nA = (t + 2) // 2
                        nB = (t + 1) // 2
                        psVa = psVa_pool.tile([_P, _D], f32, tag="psVa")
                        psVb = None
                        if nB:
                            psVb = psVb_pool.tile([_P, _D], f32, tag="psVb")
                        ia = ib = 0
                        for u in range(t + 1):
                            if u % 2 == 0:
                                dst, first, last = psVa, ia == 0, ia == nA - 1
                                ia += 1
                            else:
                                dst, first, last = psVb, ib == 0, ib == nB - 1
                                ib += 1
                            nc.tensor.matmul(
                                dst[:],
                                et_t[:, u, mcol : mcol + _P],
                                v_t[:, u * _D : (u + 1) * _D],
                                start=first, stop=last,
                            )
                        nc.vector.tensor_scalar_mul(
                            vo_t[:, ti, :], psVa[:], recip[:]
                        )
                        if nB:
                            nc.vector.scalar_tensor_tensor(
                                vo_t[:, ti, :], psVb[:], recip[:],
                                vo_t[:, ti, :], ALU.mult, ALU.add,
                            )

                    # ---- batched group DMAs (2 tiles each) ----
                    sig_dst = sig_ext[p, m0g : m0g + 2 * _P, :].rearrange(
                        "(k2 i) j -> i k2 j", k2=2
                    )
                    nc.sync.dma_start(sig_dst, sig_t[:])
                    vo_dst = vout_ext[p, m0g : m0g + 2 * _P, :].rearrange(
                        "(k2 i) d -> i k2 d", k2=2
                    )
                    nc.sync.dma_start(vo_dst, vo_t[:])
    return nc


def _get_nc():
    if _state["nc"] is None:
        nc = _build_bass()
        nc.finalize()  # Bacc.finalize -> compile(): reg alloc + wait splitting
        _state["nc"] = nc
    return _state["nc"]


def _host_prep(queries, keys, values, sigma):
    q = np.asarray(queries, dtype=np.float32)
    k = np.asarray(keys, dtype=np.float32)
    v = np.asarray(values, dtype=np.float32)
    sg_in = np.asarray(sigma, dtype=np.float32)

    qT_all = np.ascontiguousarray(q.transpose(0, 2, 3, 1)).reshape(_B * _H, _E, _L)
    kT_all = np.ascontiguousarray(k.transpose(0, 2, 3, 1)).reshape(_B * _H, _E, _L)
    v_all = np.ascontiguousarray(v.transpose(0, 2, 1, 3)).reshape(_B * _H, _L, _D)
    v_prep = np.ascontiguousarray(
        v_all.reshape(_B * _H, _NT, _P, _D).transpose(0, 2, 1, 3)
    ).reshape(_B * _H, _P, _NT * _D)
    qT_bf = qT_all.astype(_BF16)
    kT_bf = kT_all.astype(_BF16)
    v_bf = v_prep.astype(_BF16)

    sigbh = np.ascontiguousarray(sg_in.transpose(0, 2, 1)).reshape(_B * _H, _L)
    # match the reference's float32 computation exactly (3**x - 1 cancels
    # catastrophically in f32 for tiny x, so f64 here would NOT match)
    sg32 = (
        np.power(
            np.float32(3.0),
            (np.float32(1) / (np.float32(1) + np.exp(-np.float32(5) * sigbh)))
            + np.float32(1e-5),
        )
        - np.float32(1)
    ).astype(np.float32)
    sgd = sg32.astype(np.float64)
    neg_a = (-1.0 / (2.0 * sgd * sgd)).astype(np.float32)
    lnc = (-np.log(np.sqrt(2.0 * np.pi) * sgd)).astype(np.float32)
    trio = np.stack([sg32, neg_a, lnc], axis=-1).reshape(_B * _H, _NT, _P, 3)

    ii = np.arange(_P, dtype=np.int64)[:, None]
    uu = np.arange(2 * _L, dtype=np.int64)[None, :]
    gsq = ((ii + _L - uu).astype(np.float32)) ** 2

    jj = np.arange(_P)
    maskA = np.where(jj[None, :] <= jj[:, None], np.float32(0), np.float32(-1e30))
    maskA = maskA.astype(np.float32)
    maskT = np.where(jj[:, None] <= jj[None, :], np.float32(0), np.float32(-1e30))
    maskT = maskT.astype(np.float32)

    in_maps = []
    for c in range(_NCORES):
        sl = slice(_PAIRS * c, _PAIRS * (c + 1))
        in_maps.append(
            {
                "qT": np.ascontiguousarray(qT_bf[sl]),
                "kT": np.ascontiguousarray(kT_bf[sl]),
                "v": np.ascontiguousarray(v_bf[sl]),
                "vecs": np.ascontiguousarray(trio[sl].transpose(2, 0, 1, 3)),
                "gsq": gsq,
                "maskA": maskA,
                "maskT": maskT,
            }
        )
    return in_maps


def kernel(queries, keys, values, sigma):
    global LAST_EXEC_NS, LAST_RESULTS
    from concourse.bass_utils import run_bass_kernel_spmd

    in_maps = _host_prep(queries, keys, values, sigma)
    nc = _get_nc()
    try:
        res = run_bass_kernel_spmd(nc, in_maps, core_ids=list(range(_NCORES)))
    except ModuleNotFoundError:
        # BASS_TRACE set but this image lacks the NTFF profile hook module;
        # rerun untraced
        import os

        os.environ["BASS_NEVER_TRACE"] = "1"
        res = run_bass_kernel_spmd(nc, in_maps, core_ids=list(range(_NCORES)))
    LAST_RESULTS = res
    LAST_EXEC_NS = res.exec_time_ns

    ser = np.concatenate([res.results[c]["ser"] for c in range(_NCORES)], axis=0)
    pri = np.concatenate([res.results[c]["pri"] for c in range(_NCORES)], axis=0)
    sig = np.concatenate([res.results[c]["sig"] for c in range(_NCORES)], axis=0)
    vou = np.concatenate([res.results[c]["vout"] for c in range(_NCORES)], axis=0)

    series = ser.reshape(_B, _H, _L, _L).astype(np.float32)
    prior = pri.reshape(_B, _H, _L, _L).astype(np.float32)
    sigma_out = sig.reshape(_B, _H, _L, _L).astype(np.float32)
    V = np.ascontiguousarray(vou.reshape(_B, _H, _L, _D).transpose(0, 2, 1, 3))
    return (V, series, prior, sigma_out)


# revision 17
# speedup vs baseline: 1.1570x; 1.1570x over previous
"""AnomalyAttention distributed Bass kernel for 8 TRN2 NeuronCores.

Shards the 32 (batch, head) pairs across 8 cores (4 pairs/core), no
collectives. Per pair it computes causal softmax attention (series, V),
a per-row Gaussian prior, and the broadcast sigma tensor.

Layout strategy (all host-side prep is pure numpy reshuffling):
  - Q, K ship as bf16 pre-transposed to [pair, E, L] so the TensorE
    contraction dim (E=64) lands on SBUF partitions with contiguous
    DMAs; scores accumulate in f32 PSUM (fp32 matmul on TRN2 runs as
    LOW/HIGH double passes with no fast-weight-load -- 8x slower).
  - V ships as bf16 chunk-major [pair, 128, 8*64] so each 128-row
    s-chunk is a [128, 64] SBUF slice.
  - sigma is pre-transformed on host into per-row (sg, -1/(2 sg^2),
    ln(1/(sqrt(2pi) sg))) so prior is a single ScalarE
    Exp(d^2 * scale + bias) pass over a shifted window of a static
    distance^2 table.
  - series is only written on the causal half; the runtime pre-zeros
    output buffers, so the strict upper triangle stays 0. prior is a
    band matrix in f32 (sg <= 2 always), so only a 256-col diagonal
    band is computed/written per 128-row tile.
  - The P^T needed by the P@V matmul is recomputed as scores^T chunks
    (bf16 matmuls, N=256 shared across each 2-row-tile group); the
    ScalarE exp that maps them to bf16 doubles as the PSUM->SBUF copy.
    P@V accumulates into two alternating PSUM banks so consecutive
    matmuls pipeline, then is merged and normalized on VectorE.
  - series/prior/sigma_out are stored bf16 on device and upcast to f32
    on the host; V stays f32 end to end.

Measured on 8 axon-tunneled TRN2 cores: HW exec 101-115 us
(run-to-run), worst-output L2 rel err 3.2e-3 vs the f32 reference.
"""

import sys

if "/opt/trn_rl_repo" not in sys.path:
    sys.path.insert(0, "/opt/trn_rl_repo")

import ml_dtypes
import numpy as np

_B, _L, _H, _E, _D = 4, 1024, 8, 64, 64
_NCORES = 8
_PAIRS = (_B * _H) // _NCORES  # 4 (b,h) pairs per core
_P = 128
_NT = _L // _P  # 8 row tiles of 128

_BF16 = ml_dtypes.bfloat16

LAST_EXEC_NS = None
LAST_RESULTS = None

_state = {"nc": None}


def _build_bass():
    import concourse.bacc as bacc
    import concourse.mybir as mybir
    from concourse.tile import TileContext

    f32 = mybir.dt.float32
    bf16 = mybir.dt.bfloat16
    AF = mybir.ActivationFunctionType
    ALU = mybir.AluOpType

    nc = bacc.Bacc()
    qT_ext = nc.declare_dram_parameter("qT", [_PAIRS, _E, _L], bf16, isOutput=False)
    kT_ext = nc.declare_dram_parameter("kT", [_PAIRS, _E, _L], bf16, isOutput=False)
    v_ext = nc.declare_dram_parameter(
        "v", [_PAIRS, _P, _NT * _D], bf16, isOutput=False
    )
    vec_ext = nc.declare_dram_parameter(
        "vecs", [_P, _PAIRS, _NT, 3], f32, isOutput=False
    )
    g_ext = nc.declare_dram_parameter("gsq", [_P, 2 * _L], f32, isOutput=False)
    mA_ext = nc.declare_dram_parameter("maskA", [_P, _P], f32, isOutput=False)
    mT_ext = nc.declare_dram_parameter("maskT", [_P, _P], f32, isOutput=False)

    ser_ext = nc.declare_dram_parameter("ser", [_PAIRS, _L, _L], bf16, isOutput=True)
    pri_ext = nc.declare_dram_parameter("pri", [_PAIRS, _L, _L], bf16, isOutput=True)
    sig_ext = nc.declare_dram_parameter("sig", [_PAIRS, _L, _L], bf16, isOutput=True)
    vout_ext = nc.declare_dram_parameter("vout", [_PAIRS, _L, _D], f32, isOutput=True)

    with TileContext(nc) as tc:
        with (
            tc.tile_pool(name="consts", bufs=1) as consts,
            tc.tile_pool(name="pairin", bufs=2) as pairin,
            tc.tile_pool(name="work", bufs=3) as work,
            tc.tile_pool(name="etp", bufs=2) as etp,
            tc.tile_pool(name="small", bufs=4) as small,
            tc.tile_pool(name="psA", bufs=2, space="PSUM") as psA_pool,
            tc.tile_pool(name="psB", bufs=2, space="PSUM") as psB_pool,
            tc.tile_pool(name="psVa", bufs=1, space="PSUM") as psVa_pool,
            tc.tile_pool(name="psVb", bufs=1, space="PSUM") as psVb_pool,
        ):
            g_t = consts.tile([_P, 2 * _L], f32)
            nc.sync.dma_start(g_t[:], g_ext[:])
            mA_t = consts.tile([_P, _P], f32)
            nc.sync.dma_start(mA_t[:], mA_ext[:])
            mT_t = consts.tile([_P, _P], f32)
            nc.sync.dma_start(mT_t[:], mT_ext[:])
            vec_t = consts.tile([_P, _PAIRS, _NT, 3], f32)
            nc.sync.dma_start(vec_t[:], vec_ext[:])
            zb_t = consts.tile([_P, _L], bf16)
            nc.gpsimd.memset(zb_t[:], 0.0)

            for p in range(_PAIRS):
                q_t = pairin.tile([_E, _L], bf16, tag="q")
                nc.sync.dma_start(q_t[:], qT_ext[p])
                k_t = pairin.tile([_E, _L], bf16, tag="k")
                nc.sync.dma_start(k_t[:], kT_ext[p])
                v_t = pairin.tile([_P, _NT * _D], bf16, tag="v")
                nc.sync.dma_start(v_t[:], v_ext[p])

                for kk in range(_NT // 2):
                    t0, t1 = 2 * kk, 2 * kk + 1
                    m0g = t0 * _P  # row base of the 2-tile group
                    w0, w1 = (t0 + 1) * _P, (t1 + 1) * _P
                    rows0 = slice(t0 * _P, (t0 + 1) * _P)
                    rows1 = slice(t1 * _P, (t1 + 1) * _P)

                    # ---- PE block: row scores for both tiles ----
                    psA0 = psA_pool.tile([_P, _L], f32, tag="psA")
                    for n0 in range(0, w0, 512):
                        n1 = min(w0, n0 + 512)
                        nc.tensor.matmul(
                            psA0[:, n0:n1], q_t[:, rows0], k_t[:, n0:n1],
                            start=True, stop=True,
                        )
                    psA1 = psA_pool.tile([_P, _L], f32, tag="psA")
                    for n0 in range(0, w1, 512):
                        n1 = min(w1, n0 + 512)
                        nc.tensor.matmul(
                            psA1[:, n0:n1], q_t[:, rows1], k_t[:, n0:n1],
                            start=True, stop=True,
                        )

                    # ---- PE block: scores^T chunks, N=256 over both tiles
                    # (last chunk t1 is diag-only, N=128), 2 chunks per bank
                    et_t = etp.tile([_P, _NT, 2 * _P], bf16, tag="et")
                    psBs = []
                    for u0 in range(0, t1 + 1, 2):
                        psB = psB_pool.tile([_P, 4 * _P], f32, tag="psB")
                        nc.tensor.matmul(
                            psB[:, 0 : 2 * _P],
                            k_t[:, u0 * _P : (u0 + 1) * _P],
                            q_t[:, m0g : m0g + 2 * _P],
                            start=True, stop=True,
                        )
                        if u0 + 1 == t1:
                            nc.tensor.matmul(
                                psB[:, 3 * _P : 4 * _P],
                                k_t[:, t1 * _P : (t1 + 1) * _P],
                                q_t[:, rows1],
                                start=True, stop=True,
                            )
                        else:
                            nc.tensor.matmul(
                                psB[:, 2 * _P : 4 * _P],
                                k_t[:, (u0 + 1) * _P : (u0 + 2) * _P],
                                q_t[:, m0g : m0g + 2 * _P],
                                start=True, stop=True,
                            )
                        psBs.append(psB)

                    # diagonal masks: chunk t0 at slot cols 0:128 of the last
                    # bank's first slot; chunk t1 at slot cols 128:256 of its
                    # second slot
                    psB_last = psBs[-1]
                    nc.vector.tensor_add(
                        psB_last[:, 0:_P], psB_last[:, 0:_P], mT_t[:]
                    )
                    nc.vector.tensor_add(
                        psB_last[:, 3 * _P : 4 * _P],
                        psB_last[:, 3 * _P : 4 * _P],
                        mT_t[:],
                    )
                    for i, psB in enumerate(psBs):
                        nc.scalar.activation(
                            et_t[:, 2 * i : 2 * i + 2, :],
                            psB[:, : 4 * _P],
                            AF.Exp,
                            scale=0.125,
                        )

                    # ---- per-tile row path + V ----
                    sig_t = work.tile([_P, 2, _L], bf16, tag="sig")
                    s2_t = work.tile([_P, 2, _L], bf16, tag="s2")
                    vo_t = small.tile([_P, 2, _D], f32, tag="vo")
                    for ti, (t, psA, w, rows) in enumerate(
                        ((t0, psA0, w0, rows0), (t1, psA1, w1, rows1))
                    ):
                        # causal mask on the diagonal 128x128 block
                        nc.vector.tensor_add(
                            psA[:, t * _P : w], psA[:, t * _P : w], mA_t[:]
                        )
                        # exp(scale * scores) -> bf16 + per-row sums
                        eb_t = work.tile([_P, _L], bf16, tag="eb")
                        nc.scalar.activation(
                            eb_t[:, :w], psA[:, :w], AF.Exp, scale=0.125,
                        )
                        rsum = small.tile([_P, 1], f32, tag="rsum")
                        nc.vector.reduce_sum(
                            rsum[:], eb_t[:, :w], axis=mybir.AxisListType.X
                        )
                        recip = small.tile([_P, 1], f32, tag="recip")
                        nc.vector.reciprocal(recip[:], rsum[:])
                        nc.vector.tensor_scalar_mul(
                            s2_t[:, ti, :w], eb_t[:, :w], recip[:]
                        )
                        if ti == 0:
                            nc.vector.memset(s2_t[:, 0, w:w1], 0.0)

                        # prior: sg <= 3^1-1 ~= 2.0 always, so the f32 result
                        # underflows to exactly 0 for |i-j| >= 29 (in the
                        # reference too) -- only a band around the diagonal
                        # needs computing/writing (padded to 256 cols for
                        # >=512B DMA rows); pre-zeroed output covers the rest
                        m0 = t * _P
                        lo = max(0, m0 - 64)
                        hi = min(_L, lo + 2 * _P)
                        bw = hi - lo
                        pri_t = work.tile([_P, 2 * _P], bf16, tag="pri")
                        gview = g_t[:, lo - m0 + _L : hi - m0 + _L]
                        nc.scalar.activation(
                            pri_t[:, :bw], gview, AF.Exp,
                            scale=vec_t[:, p, t, 1:2],
                            bias=vec_t[:, p, t, 2:3],
                        )
                        nc.sync.dma_start(
                            pri_ext[p, rows, lo:hi], pri_t[:, :bw]
                        )

                        # sigma_out = sg broadcast along the free dim
                        nc.gpsimd.tensor_scalar(
                            sig_t[:, ti, :], zb_t[:], 0.0,
                            vec_t[:, p, t, 0:1], ALU.mult, ALU.add,
                        )

                        # V: accumulate P^T-chunk matmuls, alternating two
                        # PSUM banks so consecutive matmuls pipeline
                        mcol = ti * _P
                        nA = (t + 2) // 2
                        nB = (t + 1) // 2
                        psVa = psVa_pool.tile([_P, _D], f32, tag="psVa")
                        psVb = None
                        if nB:
                            psVb = psVb_pool.tile([_P, _D], f32, tag="psVb")
                        ia = ib = 0
                        for u in range(t + 1):
                            if u % 2 == 0:
                                dst, first, last = psVa, ia == 0, ia == nA - 1
                                ia += 1
                            else:
                                dst, first, last = psVb, ib == 0, ib == nB - 1
                                ib += 1
                            nc.tensor.matmul(
                                dst[:],
                                et_t[:, u, mcol : mcol + _P],
                                v_t[:, u * _D : (u + 1) * _D],
                                start=first, stop=last,
                            )
                        nc.vector.tensor_scalar_mul(
                            vo_t[:, ti, :], psVa[:], recip[:]
                        )
                        if nB:
                            nc.vector.scalar_tensor_tensor(
                                vo_t[:, ti, :], psVb[:], recip[:],
                                vo_t[:, ti, :], ALU.mult, ALU.add,
                            )

                    # ---- batched group DMAs (2 tiles each) ----
                    ser_dst = ser_ext[p, m0g : m0g + 2 * _P, 0:w1].rearrange(
                        "(k2 i) j -> i k2 j", k2=2
                    )
                    nc.sync.dma_start(ser_dst, s2_t[:, :, 0:w1])
                    sig_dst = sig_ext[p, m0g : m0g + 2 * _P, :].rearrange(
                        "(k2 i) j -> i k2 j", k2=2
                    )
                    nc.sync.dma_start(sig_dst, sig_t[:])
                    vo_dst = vout_ext[p, m0g : m0g + 2 * _P, :].rearrange(
                        "(k2 i) d -> i k2 d", k2=2
                    )
                    nc.sync.dma_start(vo_dst, vo_t[:])
    return nc


def _get_nc():
    if _state["nc"] is None:
        nc = _build_bass()
        nc.finalize()  # Bacc.finalize -> compile(): reg alloc + wait splitting
        _state["nc"] = nc
    return _state["nc"]


def _host_prep(queries, keys, values, sigma):
    q = np.asarray(queries, dtype=np.float32)
    k = np.asarray(keys, dtype=np.float32)
    v = np.asarray(values, dtype=np.float32)
    sg_in = np.asarray(sigma, dtype=np.float32)

    qT_all = np.ascontiguousarray(q.transpose(0, 2, 3, 1)).reshape(_B * _H, _E, _L)
    kT_all = np.ascontiguousarray(k.transpose(0, 2, 3, 1)).reshape(_B * _H, _E, _L)
    v_all = np.ascontiguousarray(v.transpose(0, 2, 1, 3)).reshape(_B * _H, _L, _D)
    v_prep = np.ascontiguousarray(
        v_all.reshape(_B * _H, _NT, _P, _D).transpose(0, 2, 1, 3)
    ).reshape(_B * _H, _P, _NT * _D)
    qT_bf = qT_all.astype(_BF16)
    kT_bf = kT_all.astype(_BF16)
    v_bf = v_prep.astype(_BF16)

    sigbh = np.ascontiguousarray(sg_in.transpose(0, 2, 1)).reshape(_B * _H, _L)
    # match the reference's float32 computation exactly (3**x - 1 cancels
    # catastrophically in f32 for tiny x, so f64 here would NOT match)
    sg32 = (
        np.power(
            np.float32(3.0),
            (np.float32(1) / (np.float32(1) + np.exp(-np.float32(5) * sigbh)))
            + np.float32(1e-5),
        )
        - np.float32(1)
    ).astype(np.float32)
    sgd = sg32.astype(np.float64)
    neg_a = (-1.0 / (2.0 * sgd * sgd)).astype(np.float32)
    lnc = (-np.log(np.sqrt(2.0 * np.pi) * sgd)).astype(np.float32)
    trio = np.stack([sg32, neg_a, lnc], axis=-1).reshape(_B * _H, _NT, _P, 3)

    ii = np.arange(_P, dtype=np.int64)[:, None]
    uu = np.arange(2 * _L, dtype=np.int64)[None, :]
    gsq = ((ii + _L - uu).astype(np.float32)) ** 2

    jj = np.arange(_P)
    maskA = np.where(jj[None, :] <= jj[:, None], np.float32(0), np.float32(-1e30))
    maskA = maskA.astype(np.float32)
    maskT = np.where(jj[:, None] <= jj[None, :], np.float32(0), np.float32(-1e30))
    maskT = maskT.astype(np.float32)

    in_maps = []
    for c in range(_NCORES):
        sl = slice(_PAIRS * c, _PAIRS * (c + 1))
        in_maps.append(
            {
                "qT": np.ascontiguousarray(qT_bf[sl]),
                "kT": np.ascontiguousarray(kT_bf[sl]),
                "v": np.ascontiguousarray(v_bf[sl]),
                "vecs": np.ascontiguousarray(trio[sl].transpose(2, 0, 1, 3)),
                "gsq": gsq,
                "maskA": maskA,
                "maskT": maskT,
            }
        )
    return in_maps


def kernel(queries, keys, values, sigma):
    global LAST_EXEC_NS, LAST_RESULTS
    from concourse.bass_utils import run_bass_kernel_spmd

    in_maps = _host_prep(queries, keys, values, sigma)
    nc = _get_nc()
    try:
        res = run_bass_kernel_spmd(nc, in_maps, core_ids=list(range(_NCORES)))
    except ModuleNotFoundError:
        # BASS_TRACE set but this image lacks the NTFF profile hook module;
        # rerun untraced
        import os

        os.environ["BASS_NEVER_TRACE"] = "1"
        res = run_bass_kernel_spmd(nc, in_maps, core_ids=list(range(_NCORES)))
    LAST_RESULTS = res
    LAST_EXEC_NS = res.exec_time_ns

    ser = np.concatenate([res.results[c]["ser"] for c in range(_NCORES)], axis=0)
    pri = np.concatenate([res.results[c]["pri"] for c in range(_NCORES)], axis=0)
    sig = np.concatenate([res.results[c]["sig"] for c in range(_NCORES)], axis=0)
    vou = np.concatenate([res.results[c]["vout"] for c in range(_NCORES)], axis=0)

    series = ser.reshape(_B, _H, _L, _L).astype(np.float32)
    prior = pri.reshape(_B, _H, _L, _L).astype(np.float32)
    sigma_out = sig.reshape(_B, _H, _L, _L).astype(np.float32)
    V = np.ascontiguousarray(vou.reshape(_B, _H, _L, _D).transpose(0, 2, 1, 3))
    return (V, series, prior, sigma_out)
